# revision 30
# baseline (speedup 1.0000x reference)
"""Trainium2 Bass kernel for nn_MultiHeadAttention_43971875177057.

MHA with residual: B=2, S=4096, d_model=512, n_heads=8, dk=64.
out = (softmax(mask(QK^T/sqrt(dk))) @ V).reshape(b,s,d) @ Wo^T + bo + x
where the reshape interleaves heads and sequence (no transpose back).

Sharding: 8 cores = (batch b in {0,1}) x (head-pair hp in {0..3}).
Each core computes attention for 2 heads of one batch over the full
sequence.  Key structure:

 * Scores are computed TRANSPOSED (S^T[k,q]) so attn comes out of the
   PE with partition=k, which feeds ctx^T = V^T @ attn^T directly.
 * The two heads' score matmuls (contraction 64) sit at base
   partitions 0/64 and are emitted back-to-back, so they run
   CONCURRENTLY on disjoint PE row-groups (~2x on the score stage,
   verified: the second MM of each pair measures ~24ns).
 * Per-head score PSUM tiles (s0/s1) give a 2-deep pipeline between
   the PE and the ACT exp; every 8th k-tile's exp runs on the DVE
   instead via a Schraudolph-style bit-trick (round(s*128/ln2 +
   16248.6) bitcast as bf16), offloading the ACT bottleneck.
 * Softmax needs no max-subtraction (scores ~ N(0,1)); the row-sum
   falls out of the ctx matmul via a ones-column appended to V;
   masking is multiplicative post-exp.
 * ctx PSUM is evacuated to SBUF right after accumulation so the
   reciprocal's DRAM-bounce partition-broadcast stays off the
   critical path; output projection reuses the ctx PSUM banks.
"""

import os
import sys
import types

import numpy as np
import ml_dtypes

B, S, D, H, DK = 2, 4096, 512, 8, 64
QC = 1024          # q-chunk (free dim of score tiles)
FS = 640           # h1 columns on the fused DVE exp+mask path
RT = S // 8        # output rows per head (the interleaved reshape)
BF16 = ml_dtypes.bfloat16

# Schraudolph exp in bf16: exp(s) ~= bitcast_bf16(u16(s*128/ln2 + B16))
EXP_A = 128.0 / 0.6931471805599453
EXP_B = 127.0 * 128.0 - 7.41


def _build_kernel(n_cores=8):
    import concourse.bacc as bacc
    import concourse.mybir as mybir
    import concourse.tile as tile
    import concourse.bass as bass

    f32 = mybir.dt.float32
    bf16 = mybir.dt.bfloat16
    u16 = mybir.dt.uint16
    C = D // 128       # 4 contraction chunks for the projections
    NKT = S // 128     # 32 key tiles
    NQC = S // QC      # 4 q chunks
    NKQ = NKT // NQC   # 8 key tiles per quarter

    nc = bacc.Bacc("TRN2", target_bir_lowering=False, debug=False,
                   num_devices=n_cores)

    i16 = mybir.dt.int16
    xT = nc.dram_tensor("xT", [D, S], bf16, kind="ExternalInput").ap()
    maskT = nc.dram_tensor("maskT", [S, S], bf16, kind="ExternalInput").ap()
    bmT = nc.dram_tensor("bmT", [S, S], i16, kind="ExternalInput").ap()
    # wo duplicated onto partitions 64-127 so the two heads' output
    # projections can row-tile onto disjoint PE row-groups.
    wo2 = nc.dram_tensor("wo2", [128, 8 * D], bf16, kind="ExternalInput").ap()
    wq = nc.dram_tensor("wq", [128, C * 128], bf16, kind="ExternalInput").ap()
    wk = nc.dram_tensor("wk", [128, C * 128], bf16, kind="ExternalInput").ap()
    wv = nc.dram_tensor("wv", [128, C * 130], bf16, kind="ExternalInput").ap()
    bqs = nc.dram_tensor("bqs", [128, 1], f32, kind="ExternalInput").ap()
    bks = nc.dram_tensor("bks", [128, 1], f32, kind="ExternalInput").ap()
    bv = nc.dram_tensor("bv", [1, 130], f32, kind="ExternalInput").ap()
    xres = nc.dram_tensor("xres", [2 * RT, D], f32, kind="ExternalInput").ap()
    out = nc.dram_tensor("out", [2 * RT, D], f32, kind="ExternalOutput").ap()
    rc_dram = nc.dram_tensor("rc_scratch", [NQC, 2, QC], bf16).ap()
    sum_dram = nc.dram_tensor("sum_scratch", [NQC, 2, QC], bf16).ap()

    Exp = mybir.ActivationFunctionType.Exp
    Mul = mybir.AluOpType.mult
    Add = mybir.AluOpType.add

    def pbcast(ap, p):
        # broadcast a [1, ...] DRAM AP along partitions
        return bass.AP(tensor=ap.tensor, offset=ap.offset,
                       ap=[[0, p]] + list(ap.ap[1:]))

    with tile.TileContext(nc) as tc:
        with (
            tc.tile_pool(name="const", bufs=1) as const,
            tc.tile_pool(name="mask", bufs=12) as maskp,
            tc.tile_pool(name="attn", bufs=6) as attnp,
            tc.tile_pool(name="outp", bufs=3) as outp,
            tc.tile_pool(name="small", bufs=4) as small,
            tc.tile_pool(name="psum", bufs=1, space="PSUM") as psum,
        ):
            # ---- constant loads (q/k weights + quarter-0 x first so
            # the projection chain starts as early as possible) -------
            wq_sb = const.tile([128, C, 128], bf16)
            nc.sync.dma_start(out=wq_sb, in_=wq.rearrange("p (c n) -> p c n", c=C))
            wk_sb = const.tile([128, C, 128], bf16)
            nc.sync.dma_start(out=wk_sb, in_=wk.rearrange("p (c n) -> p c n", c=C))
            bq_sb = const.tile([128, 1], f32)
            nc.sync.dma_start(out=bq_sb, in_=bqs)
            bk_sb = const.tile([128, 1], f32)
            nc.sync.dma_start(out=bk_sb, in_=bks)

            xT_r = xT.rearrange("(c p) s -> c p s", p=128)
            xt_c = [const.tile([128, S], bf16, tag=f"xt{c}", name=f"xt{c}")
                    for c in range(C)]
            for i in range(NQC):
                for c in range(C):
                    nc.sync.dma_start(
                        out=xt_c[c][:, i * QC:(i + 1) * QC],
                        in_=xT_r[c][:, i * QC:(i + 1) * QC])
                if i == 0:
                    wv_sb = const.tile([128, C, 130], bf16)
                    nc.sync.dma_start(
                        out=wv_sb, in_=wv.rearrange("p (c n) -> p c n", c=C))
                    bv_sb = const.tile([128, 130], f32)
                    nc.sync.dma_start(out=bv_sb, in_=pbcast(bv, 128))

            wo_sb = const.tile([128, 8, D], bf16)
            nc.sync.dma_start(out=wo_sb, in_=wo2.rearrange("c (j f) -> c j f", j=8))
            # residual (with bo folded in) preloaded: rows h*RT+qc*128
            xres_sb = const.tile([128, 8, D], f32)
            nc.sync.dma_start(out=xres_sb,
                              in_=xres.rearrange("(g p) d -> p g d", p=128))

            # ---- projections (all up front; alternate PSUM tags) -----
            qt_c = [None] * NQC
            kt_c = [None] * NQC
            v_c = [const.tile([128, NKQ, 130], bf16, tag=f"v{i}", name=f"v{i}")
                   for i in range(NQC)]

            def proj_qk(nm, w_sb, b_sb, lst, scale, i, tag):
                ps = psum.tile([128, QC], f32, tag=tag, name="pqk")
                for c in range(C):
                    for w0 in range(0, QC, 512):
                        nc.tensor.matmul(
                            ps[:, w0:w0 + 512], lhsT=w_sb[:, c, :],
                            rhs=xt_c[c][:, i * QC + w0:i * QC + w0 + 512],
                            start=(c == 0), stop=(c == C - 1))
                t = const.tile([128, QC], bf16, tag=f"{nm}{i}", name=f"{nm}{i}")
                nc.vector.tensor_scalar(t, ps, scale, b_sb, Mul, Add)
                lst[i] = t

            def proj_v(kt, tag):
                ps = psum.tile([128, 130], f32, tag=tag, name="pv")
                for c in range(C):
                    nc.tensor.matmul(ps, lhsT=xt_c[c][:, kt * 128:(kt + 1) * 128],
                                     rhs=wv_sb[:, c, :],
                                     start=(c == 0), stop=(c == C - 1))
                nc.vector.tensor_add(v_c[kt // NKQ][:, kt % NKQ, :], ps, bv_sb)

            # Q pre-scaled by EXP_A so scores arrive as A*s (the ACT
            # path divides back out via the activation's free scale).
            # All q/k projections up front; V only for quarter 0 -- the
            # other quarters' V projections are woven one-per-k-tile
            # into the first q-chunk so they overlap the attention
            # pipeline instead of serializing the prologue.
            for i in range(NQC):
                proj_qk("qt", wq_sb, bq_sb, qt_c, 0.125 * EXP_A, i, "s0")
                proj_qk("kt", wk_sb, bk_sb, kt_c, 1.0, i, "s1")
            for kt in range(NKT):
                proj_v(kt, "s0" if kt % 2 == 0 else "s1")

            # ---- attention --------------------------------------------
            # combined [128, S] ctx: head 0 in partitions 0-63, head 1
            # in 64-127, so output-projection pairs row-tile on the PE.
            ctxT2 = const.tile([128, S], bf16, tag="ctxT2", name="ctxT2")

            def emit_outproj(pqc):
                ops = psum.tile([128, 2, D], f32, tag="ctx", name="ops")
                ctx3 = ctxT2.rearrange("p (t j) -> p j t", j=8)
                for j in range(8):
                    for h in (0, 1):
                        nc.tensor.matmul(
                            ops[:, h, :],
                            lhsT=ctx3[h * 64:(h + 1) * 64, j,
                                      pqc * 128:(pqc + 1) * 128],
                            rhs=wo_sb[h * 64:(h + 1) * 64, j, :],
                            start=(j == 0), stop=(j == 7))
                for h in (0, 1):
                    r0 = h * RT + pqc * 128
                    osb = outp.tile([128, D], f32, tag="osb", name="osb")
                    nc.vector.tensor_add(osb, ops[:, h, :],
                                         xres_sb[:, h * NQC + pqc, :])
                    nc.sync.dma_start(out=out[r0:r0 + 128, :], in_=osb)

            for qc in range(NQC):
                q0 = qc * QC
                ctx_ps = psum.tile([65, 2, QC], f32, tag="ctx", name="ctx")
                at_q = []  # software pipeline: ctx(kt) emitted at kt+1

                def emit_ctx(ckt, at):
                    for h in (0, 1):
                        vt = v_c[ckt // NKQ][:, ckt % NKQ, h * 65:(h + 1) * 65]
                        for w0 in range(0, QC, 512):
                            nc.tensor.matmul(
                                ctx_ps[:, h, w0:w0 + 512], lhsT=vt,
                                rhs=at[h][:, w0:w0 + 512],
                                start=(ckt == 0), stop=(ckt == NKT - 1))

                for kt in range(NKT):
                    k0 = kt * 128
                    mt = maskp.tile([128, QC], bf16, tag="mt", name="mt")
                    nc.gpsimd.dma_start(out=mt,
                                        in_=maskT[k0:k0 + 128, q0:q0 + QC])
                    bm = maskp.tile([128, QC], i16, tag="bm", name="bm")
                    nc.gpsimd.dma_start(out=bm,
                                        in_=bmT[k0:k0 + 128, q0:q0 + QC])
                    sps = [psum.tile([128, QC], f32, tag=f"s{h}", name=f"s{h}")
                           for h in (0, 1)]
                    kq = kt_c[k0 // QC]
                    kk = k0 % QC
                    for w0 in range(0, QC, 512):
                        for h in (0, 1):
                            nc.tensor.matmul(
                                sps[h][:, w0:w0 + 512],
                                lhsT=kq[h * 64:(h + 1) * 64, kk:kk + 128],
                                rhs=qt_c[qc][h * 64:(h + 1) * 64, w0:w0 + 512],
                                start=True, stop=True)
                    at = [attnp.tile([128, QC], bf16, tag=f"at{h}",
                                     name=f"at{h}")
                          for h in (0, 1)]
                    # h1: ONE fused DVE op does exp AND mask:
                    # u16(A*s+Bm) bitcast as bf16 is Schraudolph exp;
                    # masked Bm lands in the bf16-denormal range
                    # (~1e-34) = zero weight.  h0: exact ACT exp (the
                    # free scale divides A back out) + DVE mask.
                    nc.scalar.activation(at[0], sps[0], Exp,
                                         scale=1.0 / EXP_A)
                    nc.vector.tensor_add(at[1][:, :].bitcast(u16),
                                         sps[1], bm)
                    nc.vector.tensor_mul(at[0], at[0], mt)
                    at_q.append(at)
                    if kt > 0:
                        emit_ctx(kt - 1, at_q[kt - 1])
                emit_ctx(NKT - 1, at_q[NKT - 1])

                # ---- softmax tail: evacuate PSUM fast, then bounce ----
                ctxc = outp.tile([65, 2, QC], bf16, tag="ctxc")
                # evacuate on the half-idle ACT engine (DVE is the
                # saturated engine in steady state; ScE->SBUF from PSUM
                # is also ACT's faster direction)
                nc.scalar.copy(ctxc, ctx_ps)
                # previous q-chunk's output projection: its inputs have
                # long been ready, and the ctx banks just freed -- it
                # hides inside this q-chunk's reciprocal DMA chain.
                if qc >= 1:
                    emit_outproj(qc - 1)
                nc.sync.dma_start(out=sum_dram[qc:qc + 1], in_=ctxc[64:65])
                sum_flat = sum_dram.rearrange("q a b -> q (a b)")
                sums = small.tile([128, 2 * QC // 128], bf16, tag="sums")
                nc.sync.dma_start(
                    out=sums,
                    in_=sum_flat[qc:qc + 1].rearrange("o (p f) -> (o p) f", p=128))
                rc = small.tile([128, 2 * QC // 128], bf16, tag="rc")
                with nc.allow_low_precision(reason="softmax 1/sum in bf16"):
                    nc.vector.reciprocal(rc, sums)
                rc_flat = rc_dram.rearrange("q a b -> q (a b)")
                nc.sync.dma_start(
                    out=rc_flat[qc:qc + 1].rearrange("o (p f) -> (o p) f", p=128),
                    in_=rc)
                rcr = small.tile([64, 2, QC], bf16, tag="rcr")
                nc.sync.dma_start(out=rcr, in_=pbcast(rc_dram[qc:qc + 1], 64))
                # h0 lands on partitions 0-63 directly; h1 is normalized
                # on partitions 0-63 then DMA-shifted to partitions
                # 64-127 (engines cannot move data across partitions).
                nc.vector.tensor_mul(ctxT2[0:64, q0:q0 + QC],
                                     ctxc[0:64, 0, :], rcr[:, 0, :])
                ch1 = outp.tile([64, QC], bf16, tag="ch1", name="ch1")
                nc.vector.tensor_mul(ch1, ctxc[0:64, 1, :], rcr[:, 1, :])
                nc.sync.dma_start(out=ctxT2[64:128, q0:q0 + QC], in_=ch1)

            emit_outproj(NQC - 1)

    nc.compile()
    return nc


def _shard_inputs(x, mask, Wq, bq, Wk, bk, Wv, bv, Wo, bo):
    """Host-side marshaling: slice/transpose/cast per core. core = b*4+hp."""
    C = D // 128
    keep = (1 - mask[0, 0]).astype(np.int16)
    keepT = np.ascontiguousarray(keep.T).astype(BF16)
    # Fused exp+mask bias: keep -> round(EXP_B)=16249 (Schraudolph B),
    # masked -> 8192 (A*s + 8192 stays in [6346, 18095]: positive, and
    # masked values bitcast to bf16 denormals ~1e-34 = zero weight).
    bmT = np.ascontiguousarray(
        (keep.T * np.int16(16249 - 8192) + np.int16(8192)).astype(np.int16))
    woT = Wo.T.astype(np.float32)
    wo_re = np.ascontiguousarray(
        woT.reshape(8, 64, D).transpose(1, 0, 2).reshape(64, 8 * D)).astype(BF16)

    def re_w(wT):
        # [D, n] -> [128, C*n]  with  out[p, c*n+j] = wT[c*128+p, j]
        n = wT.shape[1]
        return np.ascontiguousarray(
            wT.reshape(C, 128, n).transpose(1, 0, 2).reshape(128, C * n)
        ).astype(BF16)

    in_maps = []
    for core in range(8):
        b, hp = divmod(core, 4)
        c0 = hp * 128
        wvT_ext = np.zeros((D, 130), np.float32)
        wvT_ext[:, 0:64] = Wv[c0:c0 + 64, :].T
        wvT_ext[:, 65:129] = Wv[c0 + 64:c0 + 128, :].T
        bv_ext = np.zeros((1, 130), np.float32)
        bv_ext[0, 0:64] = bv[c0:c0 + 64]
        bv_ext[0, 64] = 1.0
        bv_ext[0, 65:129] = bv[c0 + 64:c0 + 128]
        bv_ext[0, 129] = 1.0
        # residual with the output bias folded in host-side
        xr = x[b, hp * 2 * RT:(hp + 1) * 2 * RT, :] + bo.reshape(1, D)
        in_maps.append({
            "xT": np.ascontiguousarray(x[b].T).astype(BF16),
            "maskT": keepT,
            "bmT": bmT,
            "wq": re_w(np.ascontiguousarray(Wq[c0:c0 + 128, :].T)),
            "wk": re_w(np.ascontiguousarray(Wk[c0:c0 + 128, :].T)),
            "wv": re_w(wvT_ext),
            "wo2": np.concatenate([wo_re, wo_re], axis=0),
            "bqs": (bq[c0:c0 + 128] * (EXP_A / 8.0)).reshape(128, 1)
                   .astype(np.float32),
            "bks": bk[c0:c0 + 128].reshape(128, 1).astype(np.float32),
            "bv": bv_ext,
            "xres": np.ascontiguousarray(xr).astype(np.float32),
        })
    return in_maps


_RESULT_CACHE = {}


def _ensure_env():
    """Make concourse importable and register the NTFF profile hook."""
    for p in ("/root/.axon_site/_ro/trn_rl_repo", "/opt/trn_rl_repo"):
        if os.path.isdir(p) and p not in sys.path:
            sys.path.append(p)
    try:
        import antenv  # noqa: F401
        import antenv.axon_hooks  # noqa: F401
    except ImportError:
        try:
            import antenv
            mod = types.ModuleType("antenv.axon_hooks")
            _hook = [None]
            mod.set_axon_ntff_profile_hook = lambda h: _hook.__setitem__(0, h)
            mod.get_axon_ntff_profile_hook = lambda: _hook[0]
            sys.modules["antenv.axon_hooks"] = mod
            antenv.axon_hooks = mod
            from trn_agent_boot.trn_boot import _ntff_profile_via_ctypes
            so = "/opt/axon/libaxon_pjrt.so"
            if os.path.exists(so):
                mod.set_axon_ntff_profile_hook(_ntff_profile_via_ctypes(so))
        except Exception:
            pass


def kernel(x, mask, Wq, bq, Wk, bk, Wv, bv, Wo, bo, trace=False):
    _ensure_env()
    from concourse.bass_utils import run_bass_kernel_spmd

    x = np.asarray(x, np.float32)
    mask = np.asarray(mask)
    args = [np.asarray(a, np.float32) for a in (Wq, bq, Wk, bk, Wv, bv, Wo, bo)]
    nc = _RESULT_CACHE.get("nc")
    if nc is None:
        nc = _build_kernel()
        _RESULT_CACHE["nc"] = nc
    in_maps = _shard_inputs(x, mask, *args)
    res = run_bass_kernel_spmd(nc, in_maps, core_ids=list(range(8)),
                               trace=trace)
    _RESULT_CACHE["last_run"] = res
    out = np.empty((B, S, D), np.float32)
    for core in range(8):
        b, hp = divmod(core, 4)
        out[b, hp * 2 * RT:(hp + 1) * 2 * RT, :] = res.results[core]["out"]
    return out


if __name__ == "__main__":
    _ensure_env()
    nc = _build_kernel()
    print("kernel built + compiled OK")


# revision 31
# speedup vs baseline: 1.0188x; 1.0188x over previous
"""Trainium2 Bass kernel for nn_MultiHeadAttention_43971875177057.

MHA with residual: B=2, S=4096, d_model=512, n_heads=8, dk=64.
out = (softmax(mask(QK^T/sqrt(dk))) @ V).reshape(b,s,d) @ Wo^T + bo + x
where the reshape interleaves heads and sequence (no transpose back).

Sharding: 8 cores = (batch b in {0,1}) x (head-pair hp in {0..3}).
Each core computes attention for 2 heads of one batch over the full
sequence.  Key structure:

 * Scores are computed TRANSPOSED (S^T[k,q]) so attn comes out of the
   PE with partition=k, which feeds ctx^T = V^T @ attn^T directly.
 * The two heads' score matmuls (contraction 64) sit at base
   partitions 0/64 and are emitted back-to-back, so they run
   CONCURRENTLY on disjoint PE row-groups (~2x on the score stage,
   verified: the second MM of each pair measures ~24ns).
 * Per-head score PSUM tiles (s0/s1) give a 2-deep pipeline between
   the PE and the ACT exp; every 8th k-tile's exp runs on the DVE
   instead via a Schraudolph-style bit-trick (round(s*128/ln2 +
   16248.6) bitcast as bf16), offloading the ACT bottleneck.
 * Softmax needs no max-subtraction (scores ~ N(0,1)); the row-sum
   falls out of the ctx matmul via a ones-column appended to V;
   masking is multiplicative post-exp.
 * ctx PSUM is evacuated to SBUF right after accumulation so the
   reciprocal's DRAM-bounce partition-broadcast stays off the
   critical path; output projection reuses the ctx PSUM banks.
"""

import os
import sys
import types

import numpy as np
import ml_dtypes

B, S, D, H, DK = 2, 4096, 512, 8, 64
QC = 1024          # q-chunk (free dim of score tiles)
FS = 640           # h1 columns on the fused DVE exp+mask path
RT = S // 8        # output rows per head (the interleaved reshape)
BF16 = ml_dtypes.bfloat16

# Schraudolph exp in bf16: exp(s) ~= bitcast_bf16(u16(s*128/ln2 + B16))
EXP_A = 128.0 / 0.6931471805599453
EXP_B = 127.0 * 128.0 - 7.41


def _build_kernel(n_cores=8):
    import concourse.bacc as bacc
    import concourse.mybir as mybir
    import concourse.tile as tile
    import concourse.bass as bass

    f32 = mybir.dt.float32
    bf16 = mybir.dt.bfloat16
    u16 = mybir.dt.uint16
    C = D // 128       # 4 contraction chunks for the projections
    NKT = S // 128     # 32 key tiles
    NQC = S // QC      # 4 q chunks
    NKQ = NKT // NQC   # 8 key tiles per quarter

    nc = bacc.Bacc("TRN2", target_bir_lowering=False, debug=False,
                   num_devices=n_cores)

    i16 = mybir.dt.int16
    xT = nc.dram_tensor("xT", [D, S], bf16, kind="ExternalInput").ap()
    maskT = nc.dram_tensor("maskT", [S, S], bf16, kind="ExternalInput").ap()
    bmT = nc.dram_tensor("bmT", [S, S], i16, kind="ExternalInput").ap()
    # wo duplicated onto partitions 64-127 so the two heads' output
    # projections can row-tile onto disjoint PE row-groups.
    wo2 = nc.dram_tensor("wo2", [128, 8 * D], bf16, kind="ExternalInput").ap()
    wq = nc.dram_tensor("wq", [128, C * 128], bf16, kind="ExternalInput").ap()
    wk = nc.dram_tensor("wk", [128, C * 128], bf16, kind="ExternalInput").ap()
    wv = nc.dram_tensor("wv", [128, C * 130], bf16, kind="ExternalInput").ap()
    bqs = nc.dram_tensor("bqs", [128, 1], f32, kind="ExternalInput").ap()
    bks = nc.dram_tensor("bks", [128, 1], f32, kind="ExternalInput").ap()
    bv = nc.dram_tensor("bv", [1, 130], f32, kind="ExternalInput").ap()
    xres = nc.dram_tensor("xres", [2 * RT, D], f32, kind="ExternalInput").ap()
    out = nc.dram_tensor("out", [2 * RT, D], f32, kind="ExternalOutput").ap()
    rc_dram = nc.dram_tensor("rc_scratch", [NQC, 2, QC], bf16).ap()
    sum_dram = nc.dram_tensor("sum_scratch", [NQC, 2, QC], bf16).ap()

    Exp = mybir.ActivationFunctionType.Exp
    Mul = mybir.AluOpType.mult
    Add = mybir.AluOpType.add

    def pbcast(ap, p):
        # broadcast a [1, ...] DRAM AP along partitions
        return bass.AP(tensor=ap.tensor, offset=ap.offset,
                       ap=[[0, p]] + list(ap.ap[1:]))

    with tile.TileContext(nc) as tc:
        with (
            tc.tile_pool(name="const", bufs=1) as const,
            tc.tile_pool(name="mask", bufs=10) as maskp,
            tc.tile_pool(name="attn", bufs=5) as attnp,
            tc.tile_pool(name="outp", bufs=2) as outp,
            tc.tile_pool(name="small", bufs=2) as small,
            tc.tile_pool(name="psum", bufs=1, space="PSUM") as psum,
        ):
            # ---- constant loads (q/k weights + quarter-0 x first so
            # the projection chain starts as early as possible) -------
            wq_sb = const.tile([128, C, 128], bf16)
            nc.sync.dma_start(out=wq_sb, in_=wq.rearrange("p (c n) -> p c n", c=C))
            wk_sb = const.tile([128, C, 128], bf16)
            nc.sync.dma_start(out=wk_sb, in_=wk.rearrange("p (c n) -> p c n", c=C))
            bq_sb = const.tile([128, 1], f32)
            nc.sync.dma_start(out=bq_sb, in_=bqs)
            bk_sb = const.tile([128, 1], f32)
            nc.sync.dma_start(out=bk_sb, in_=bks)

            xT_r = xT.rearrange("(c p) s -> c p s", p=128)
            xt_c = [const.tile([128, S], bf16, tag=f"xt{c}", name=f"xt{c}")
                    for c in range(C)]
            for i in range(NQC):
                for c in range(C):
                    nc.sync.dma_start(
                        out=xt_c[c][:, i * QC:(i + 1) * QC],
                        in_=xT_r[c][:, i * QC:(i + 1) * QC])
                if i == 0:
                    wv_sb = const.tile([128, C, 130], bf16)
                    nc.sync.dma_start(
                        out=wv_sb, in_=wv.rearrange("p (c n) -> p c n", c=C))
                    bv_sb = const.tile([128, 130], f32)
                    nc.sync.dma_start(out=bv_sb, in_=pbcast(bv, 128))

            wo_sb = const.tile([128, 8, D], bf16)
            nc.sync.dma_start(out=wo_sb, in_=wo2.rearrange("c (j f) -> c j f", j=8))
            # residual (with bo folded in) preloaded: rows h*RT+qc*128
            xres_sb = const.tile([128, 8, D], f32)
            nc.sync.dma_start(out=xres_sb,
                              in_=xres.rearrange("(g p) d -> p g d", p=128))

            # ---- projections (all up front; alternate PSUM tags) -----
            qt_c = [None] * NQC
            kt_c = [None] * NQC
            v_c = [const.tile([128, NKQ, 130], bf16, tag=f"v{i}", name=f"v{i}")
                   for i in range(NQC)]

            def proj_qk(nm, w_sb, b_sb, lst, scale, i, tag):
                ps = psum.tile([128, QC], f32, tag=tag, name="pqk")
                for c in range(C):
                    for w0 in range(0, QC, 512):
                        nc.tensor.matmul(
                            ps[:, w0:w0 + 512], lhsT=w_sb[:, c, :],
                            rhs=xt_c[c][:, i * QC + w0:i * QC + w0 + 512],
                            start=(c == 0), stop=(c == C - 1))
                t = const.tile([128, QC], bf16, tag=f"{nm}{i}", name=f"{nm}{i}")
                nc.vector.tensor_scalar(t, ps, scale, b_sb, Mul, Add)
                lst[i] = t

            def proj_v(kt, tag):
                ps = psum.tile([128, 130], f32, tag=tag, name="pv")
                for c in range(C):
                    nc.tensor.matmul(ps, lhsT=xt_c[c][:, kt * 128:(kt + 1) * 128],
                                     rhs=wv_sb[:, c, :],
                                     start=(c == 0), stop=(c == C - 1))
                nc.vector.tensor_add(v_c[kt // NKQ][:, kt % NKQ, :], ps, bv_sb)

            # Q pre-scaled by EXP_A so scores arrive as A*s (the ACT
            # path divides back out via the activation's free scale).
            # All q/k projections up front; V only for quarter 0 -- the
            # other quarters' V projections are woven one-per-k-tile
            # into the first q-chunk so they overlap the attention
            # pipeline instead of serializing the prologue.
            for i in range(NQC):
                proj_qk("qt", wq_sb, bq_sb, qt_c, 0.125 * EXP_A, i, "s0")
                proj_qk("kt", wk_sb, bk_sb, kt_c, 1.0, i, "s1")
            for kt in range(NKT):
                proj_v(kt, "s0" if kt % 2 == 0 else "s1")

            # ---- attention --------------------------------------------
            # combined [128, S] ctx: head 0 in partitions 0-63, head 1
            # in 64-127, so output-projection pairs row-tile on the PE.
            ctxT2 = const.tile([128, S], bf16, tag="ctxT2", name="ctxT2")

            def emit_outproj(pqc):
                ops = psum.tile([128, 2, D], f32, tag="ctx", name="ops")
                ctx3 = ctxT2.rearrange("p (t j) -> p j t", j=8)
                for j in range(8):
                    for h in (0, 1):
                        nc.tensor.matmul(
                            ops[:, h, :],
                            lhsT=ctx3[h * 64:(h + 1) * 64, j,
                                      pqc * 128:(pqc + 1) * 128],
                            rhs=wo_sb[h * 64:(h + 1) * 64, j, :],
                            start=(j == 0), stop=(j == 7))
                for h in (0, 1):
                    r0 = h * RT + pqc * 128
                    osb = outp.tile([128, D], f32, tag="osb", name="osb")
                    nc.vector.tensor_add(osb, ops[:, h, :],
                                         xres_sb[:, h * NQC + pqc, :])
                    nc.sync.dma_start(out=out[r0:r0 + 128, :], in_=osb)

            for qc in range(NQC):
                q0 = qc * QC
                ctx_ps = psum.tile([65, 2, QC], f32, tag="ctx", name="ctx")
                at_q = []  # software pipeline: ctx(kt) emitted at kt+1

                def emit_ctx(ckt, at):
                    for h in (0, 1):
                        vt = v_c[ckt // NKQ][:, ckt % NKQ, h * 65:(h + 1) * 65]
                        for w0 in range(0, QC, 512):
                            nc.tensor.matmul(
                                ctx_ps[:, h, w0:w0 + 512], lhsT=vt,
                                rhs=at[h][:, w0:w0 + 512],
                                start=(ckt == 0), stop=(ckt == NKT - 1))

                for kt in range(NKT):
                    k0 = kt * 128
                    mt = maskp.tile([128, QC], bf16, tag="mt", name="mt")
                    nc.gpsimd.dma_start(out=mt,
                                        in_=maskT[k0:k0 + 128, q0:q0 + QC])
                    bm = maskp.tile([128, QC], i16, tag="bm", name="bm")
                    nc.gpsimd.dma_start(out=bm,
                                        in_=bmT[k0:k0 + 128, q0:q0 + QC])
                    sps = [psum.tile([128, QC], f32, tag=f"s{h}", name=f"s{h}")
                           for h in (0, 1)]
                    kq = kt_c[k0 // QC]
                    kk = k0 % QC
                    for w0 in range(0, QC, 512):
                        for h in (0, 1):
                            nc.tensor.matmul(
                                sps[h][:, w0:w0 + 512],
                                lhsT=kq[h * 64:(h + 1) * 64, kk:kk + 128],
                                rhs=qt_c[qc][h * 64:(h + 1) * 64, w0:w0 + 512],
                                start=True, stop=True)
                    at = [attnp.tile([128, QC], bf16, tag=f"at{h}",
                                     name=f"at{h}")
                          for h in (0, 1)]
                    # h1: ONE fused DVE op does exp AND mask:
                    # u16(A*s+Bm) bitcast as bf16 is Schraudolph exp;
                    # masked Bm lands in the bf16-denormal range
                    # (~1e-34) = zero weight.  h0: exact ACT exp (the
                    # free scale divides A back out) + DVE mask.
                    nc.scalar.activation(at[0], sps[0], Exp,
                                         scale=1.0 / EXP_A)
                    nc.vector.tensor_add(at[1][:, :].bitcast(u16),
                                         sps[1], bm)
                    nc.vector.tensor_mul(at[0], at[0], mt)
                    at_q.append(at)
                    if kt > 0:
                        emit_ctx(kt - 1, at_q[kt - 1])
                emit_ctx(NKT - 1, at_q[NKT - 1])

                # ---- softmax tail: evacuate PSUM fast, then bounce ----
                ctxc = outp.tile([65, 2, QC], bf16, tag="ctxc")
                # evacuate on the half-idle ACT engine (DVE is the
                # saturated engine in steady state; ScE->SBUF from PSUM
                # is also ACT's faster direction)
                nc.scalar.copy(ctxc, ctx_ps)
                # previous q-chunk's output projection: its inputs have
                # long been ready, and the ctx banks just freed -- it
                # hides inside this q-chunk's reciprocal DMA chain.
                if qc >= 1:
                    emit_outproj(qc - 1)
                nc.sync.dma_start(out=sum_dram[qc:qc + 1], in_=ctxc[64:65])
                sum_flat = sum_dram.rearrange("q a b -> q (a b)")
                sums = small.tile([128, 2 * QC // 128], bf16, tag="sums")
                nc.sync.dma_start(
                    out=sums,
                    in_=sum_flat[qc:qc + 1].rearrange("o (p f) -> (o p) f", p=128))
                rc = small.tile([128, 2 * QC // 128], bf16, tag="rc")
                with nc.allow_low_precision(reason="softmax 1/sum in bf16"):
                    nc.vector.reciprocal(rc, sums)
                rc_flat = rc_dram.rearrange("q a b -> q (a b)")
                nc.sync.dma_start(
                    out=rc_flat[qc:qc + 1].rearrange("o (p f) -> (o p) f", p=128),
                    in_=rc)
                rcr = small.tile([64, 2, QC], bf16, tag="rcr")
                nc.sync.dma_start(out=rcr, in_=pbcast(rc_dram[qc:qc + 1], 64))
                # h0 lands on partitions 0-63 directly; h1 is normalized
                # on partitions 0-63 then DMA-shifted to partitions
                # 64-127 (engines cannot move data across partitions).
                nc.vector.tensor_mul(ctxT2[0:64, q0:q0 + QC],
                                     ctxc[0:64, 0, :], rcr[:, 0, :])
                ch1 = outp.tile([64, QC], bf16, tag="ch1", name="ch1")
                nc.vector.tensor_mul(ch1, ctxc[0:64, 1, :], rcr[:, 1, :])
                nc.sync.dma_start(out=ctxT2[64:128, q0:q0 + QC], in_=ch1)

            emit_outproj(NQC - 1)

    nc.compile()
    return nc


def _shard_inputs(x, mask, Wq, bq, Wk, bk, Wv, bv, Wo, bo):
    """Host-side marshaling: slice/transpose/cast per core. core = b*4+hp."""
    C = D // 128
    keep = (1 - mask[0, 0]).astype(np.int16)
    keepT = np.ascontiguousarray(keep.T).astype(BF16)
    # Fused exp+mask bias: keep -> round(EXP_B)=16249 (Schraudolph B),
    # masked -> 8192 (A*s + 8192 stays in [6346, 18095]: positive, and
    # masked values bitcast to bf16 denormals ~1e-34 = zero weight).
    bmT = np.ascontiguousarray(
        (keep.T * np.int16(16249 - 8192) + np.int16(8192)).astype(np.int16))
    woT = Wo.T.astype(np.float32)
    wo_re = np.ascontiguousarray(
        woT.reshape(8, 64, D).transpose(1, 0, 2).reshape(64, 8 * D)).astype(BF16)

    def re_w(wT):
        # [D, n] -> [128, C*n]  with  out[p, c*n+j] = wT[c*128+p, j]
        n = wT.shape[1]
        return np.ascontiguousarray(
            wT.reshape(C, 128, n).transpose(1, 0, 2).reshape(128, C * n)
        ).astype(BF16)

    in_maps = []
    for core in range(8):
        b, hp = divmod(core, 4)
        c0 = hp * 128
        wvT_ext = np.zeros((D, 130), np.float32)
        wvT_ext[:, 0:64] = Wv[c0:c0 + 64, :].T
        wvT_ext[:, 65:129] = Wv[c0 + 64:c0 + 128, :].T
        bv_ext = np.zeros((1, 130), np.float32)
        bv_ext[0, 0:64] = bv[c0:c0 + 64]
        bv_ext[0, 64] = 1.0
        bv_ext[0, 65:129] = bv[c0 + 64:c0 + 128]
        bv_ext[0, 129] = 1.0
        # residual with the output bias folded in host-side
        xr = x[b, hp * 2 * RT:(hp + 1) * 2 * RT, :] + bo.reshape(1, D)
        in_maps.append({
            "xT": np.ascontiguousarray(x[b].T).astype(BF16),
            "maskT": keepT,
            "bmT": bmT,
            "wq": re_w(np.ascontiguousarray(Wq[c0:c0 + 128, :].T)),
            "wk": re_w(np.ascontiguousarray(Wk[c0:c0 + 128, :].T)),
            "wv": re_w(wvT_ext),
            "wo2": np.concatenate([wo_re, wo_re], axis=0),
            "bqs": (bq[c0:c0 + 128] * (EXP_A / 8.0)).reshape(128, 1)
                   .astype(np.float32),
            "bks": bk[c0:c0 + 128].reshape(128, 1).astype(np.float32),
            "bv": bv_ext,
            "xres": np.ascontiguousarray(xr).astype(np.float32),
        })
    return in_maps


_RESULT_CACHE = {}


def _ensure_env():
    """Make concourse importable and register the NTFF profile hook."""
    for p in ("/root/.axon_site/_ro/trn_rl_repo", "/opt/trn_rl_repo"):
        if os.path.isdir(p) and p not in sys.path:
            sys.path.append(p)
    try:
        import antenv  # noqa: F401
        import antenv.axon_hooks  # noqa: F401
    except ImportError:
        try:
            import antenv
            mod = types.ModuleType("antenv.axon_hooks")
            _hook = [None]
            mod.set_axon_ntff_profile_hook = lambda h: _hook.__setitem__(0, h)
            mod.get_axon_ntff_profile_hook = lambda: _hook[0]
            sys.modules["antenv.axon_hooks"] = mod
            antenv.axon_hooks = mod
            from trn_agent_boot.trn_boot import _ntff_profile_via_ctypes
            so = "/opt/axon/libaxon_pjrt.so"
            if os.path.exists(so):
                mod.set_axon_ntff_profile_hook(_ntff_profile_via_ctypes(so))
        except Exception:
            pass


def kernel(x, mask, Wq, bq, Wk, bk, Wv, bv, Wo, bo, trace=False):
    _ensure_env()
    from concourse.bass_utils import run_bass_kernel_spmd

    x = np.asarray(x, np.float32)
    mask = np.asarray(mask)
    args = [np.asarray(a, np.float32) for a in (Wq, bq, Wk, bk, Wv, bv, Wo, bo)]
    nc = _RESULT_CACHE.get("nc")
    if nc is None:
        nc = _build_kernel()
        _RESULT_CACHE["nc"] = nc
    in_maps = _shard_inputs(x, mask, *args)
    res = run_bass_kernel_spmd(nc, in_maps, core_ids=list(range(8)),
                               trace=trace)
    _RESULT_CACHE["last_run"] = res
    out = np.empty((B, S, D), np.float32)
    for core in range(8):
        b, hp = divmod(core, 4)
        out[b, hp * 2 * RT:(hp + 1) * 2 * RT, :] = res.results[core]["out"]
    return out


if __name__ == "__main__":
    _ensure_env()
    nc = _build_kernel()
    print("kernel built + compiled OK")
